# revision 20
# baseline (speedup 1.0000x reference)
"""DSNT + JSD + distance double loss on 8 TRN2 NeuronCores.

Data-parallel: batch 64 is split into 8 shards of 8 batches; each core
computes its partial sum s_i over its 16 (b,c) heatmap slices; the host
sums the 8 partials and divides by B.

Per (b,c) slice (512x512 -> SBUF [128, 2048], partition p holds rows
h in {4p..4p+3}):
  ACT:  e = exp(x) (bf16), accum -> per-partition rowsums -> S, 1/S
  PE:   cols2 = [ones; ys]^T @ e  (bf16)  -> px_u, py_u via Pool stt
  DVE/Pool: m2 = (e*invS) + t (bf16), accum -> sum(m2)
  ACT:  l = ln(m2) (bf16), written into the same tile as m2
  PE:   gram-diag trick: for each 128-col block, matmul with
        lhsT=m2_block, rhs=[m2_block | l_block] accumulated into one
        whole-run PSUM [128,256]; its diagonals give sum(m2^2), sum(m2*l)
  argmax(t) chunked: DVE chunk-max [P,16,128]->cm, max8+max_index on cm
        -> first chunk; indirect DMA gathers that 128-wide window from
        DRAM; max_index on the window -> exact first-occurrence index.
jsd total = [0.5*W1 - 0.5*ln2*W0 - 0.25*W2] / (H*W)
"""

import math
import os

import numpy as np

import concourse.bacc as bacc
import concourse.bass as bass
import concourse.mybir as mybir
import concourse.tile as tile
from concourse.bass import IndirectOffsetOnAxis
from concourse.bass_utils import run_bass_kernel_spmd

F32 = mybir.dt.float32
BF16 = mybir.dt.bfloat16
U32 = mybir.dt.uint32
I32 = mybir.dt.int32
ALU = mybir.AluOpType
ACTF = mybir.ActivationFunctionType
AX = mybir.AxisListType

B, C, H, W = 64, 2, 512, 512
N_CORES = 8
B_SH = B // N_CORES          # 8 batches per core
NSL = B_SH * C               # 16 slices per core
P = 128                      # SBUF partitions
FD = (H * W) // P            # 2048 free elements per partition
SUB = W                      # 512-wide sub-columns (4 per row)
NSUB = FD // SUB             # 4
NCH = 16                     # argmax chunks per partition
CHW = FD // NCH              # 128 elements per chunk
NBLK = FD // P               # 16 gram blocks of 128 columns

# Input DMA buffer depth.
IN_BUFS = int(os.environ.get("K_IN_BUFS", "5"))

_CACHE = {}
LAST_RESULTS = None


def _constants():
    # cf32 [128, 260]: col0 = p*2048 (flat partition base), col1 = ones,
    # cols 2:130 = identity, col130 = p*16 (gather row base),
    # cols 132:260 = all-ones [128,128] (S broadcast matmul lhsT)
    cf32 = np.zeros((P, 260), dtype=np.float32)
    cf32[:, 0] = np.arange(P, dtype=np.float32) * FD
    cf32[:, 1] = 1.0
    cf32[:, 2:130] = np.eye(P, dtype=np.float32)
    cf32[:, 130] = np.arange(P, dtype=np.float32) * NCH
    cf32[:, 132:260] = 1.0
    # csml [2, 640]: [:, 0:512] = {xs row, ones row}; [0, 512:640] = onesr
    xs = (np.arange(W, dtype=np.float32) + 1.0) / W
    csml = np.zeros((2, 640), dtype=np.float32)
    csml[0, 0:W] = xs
    csml[1, 0:W] = 1.0
    csml[0, W:W + P] = 1.0
    # cbf [128, 8] bf16: col 2j = ones, col 2j+1 = ys_j ((4p+j+1)/512)
    import ml_dtypes
    hidx = (np.arange(P, dtype=np.float32)[:, None] * NSUB
            + np.arange(NSUB, dtype=np.float32)[None, :])
    ys = (hidx + 1.0) / H                                   # [128, 4]
    cbf = np.zeros((P, 8), dtype=np.float32)
    for j in range(NSUB):
        cbf[:, 2 * j] = 1.0
        cbf[:, 2 * j + 1] = ys[:, j]
    return {"cf32": cf32, "csml": csml,
            "cbf": cbf.astype(ml_dtypes.bfloat16)}


def _patch_act_tables():
    """Steer the act-table chooser so Exp/Ln/Copy/Identity all live in the
    single `natural_log_exp_and_others` set — otherwise the per-slice
    Exp->Ln alternation reloads tables (~1.3us each, 32x per core)."""
    if _CACHE.get("act_patched"):
        return
    import concourse.hw_specs as hw_specs

    orig = hw_specs.get_activation_tables
    hot = {ACTF.Exp, ACTF.Ln, ACTF.Square, ACTF.Copy, ACTF.Identity}

    def patched(module_arch):
        tabs = orig(module_arch)
        out = {}
        for name, funcs in tabs.items():
            if name == "natural_log_exp_and_others":
                out[name] = set(funcs)
            else:
                out[name] = set(funcs) - hot
        return out

    hw_specs.get_activation_tables = patched
    bacc.get_activation_tables = patched
    _CACHE["act_patched"] = True


def build_program():
    """Build (once) the single-core Bass/Tile program run SPMD on 8 cores."""
    if "nc" in _CACHE:
        return _CACHE["nc"]

    _patch_act_tables()
    nc = bacc.Bacc("TRN2", target_bir_lowering=False, debug=False,
                   num_devices=N_CORES)

    x_t = nc.dram_tensor("x", [NSL, P, FD], F32, kind="ExternalInput")
    t_t = nc.dram_tensor("t", [NSL, P, FD], F32, kind="ExternalInput")
    cf_d = nc.dram_tensor("cf32", [P, 260], F32, kind="ExternalInput").ap()
    cs_d = nc.dram_tensor("csml", [2, 640], F32, kind="ExternalInput").ap()
    cb_d = nc.dram_tensor("cbf", [P, 8], BF16, kind="ExternalInput").ap()
    out_d = nc.dram_tensor("out", [1, 1], F32, kind="ExternalOutput").ap()

    with tile.TileContext(nc) as tc:
        _emit(nc, tc, x_t.ap(), t_t.ap(), cf_d, cs_d, cb_d, out_d)

    nc.compile()
    _CACHE["nc"] = nc
    return nc


def _emit(nc, tc, x_d, t_d, cf_d, cs_d, cb_d, out_d):
    from contextlib import ExitStack
    ctx = ExitStack()
    with ctx:
        singles = ctx.enter_context(tc.tile_pool(name="singles", bufs=1))
        xp = ctx.enter_context(tc.tile_pool(name="xp", bufs=IN_BUFS))
        tp = ctx.enter_context(tc.tile_pool(name="tp", bufs=IN_BUFS + 3))
        ep = ctx.enter_context(tc.tile_pool(name="ep", bufs=6))
        mlp = ctx.enter_context(tc.tile_pool(name="mlp", bufs=3))
        winp = ctx.enter_context(tc.tile_pool(name="winp", bufs=4))
        sm = ctx.enter_context(tc.tile_pool(name="sm", bufs=8))
        pcols = ctx.enter_context(
            tc.tile_pool(name="pcols", bufs=3, space="PSUM"))
        pboth = ctx.enter_context(
            tc.tile_pool(name="pboth", bufs=1, space="PSUM"))
        psb = ctx.enter_context(
            tc.tile_pool(name="psb", bufs=2, space="PSUM"))
        pbig = ctx.enter_context(
            tc.tile_pool(name="pbig", bufs=1, space="PSUM"))

        # ---- constants: 3 packed DMAs on the ACT queue ----
        cf_sb = singles.tile([P, 260], F32)
        nc.scalar.dma_start(out=cf_sb, in_=cf_d)
        cs_sb = singles.tile([2, 640], F32)
        nc.scalar.dma_start(out=cs_sb, in_=cs_d)
        cb_sb = singles.tile([P, 8], BF16)
        nc.scalar.dma_start(out=cb_sb, in_=cb_d)
        pb_sb = cf_sb[:, 0:1]            # p*2048
        ones_sb = cf_sb[:, 1:2]          # 1.0 column (f32)
        eye_sb = cf_sb[:, 2:2 + P]       # identity
        p16_sb = cf_sb[:, 130:131]       # p*16
        ones128_sb = cf_sb[:, 132:260]   # all-ones [128,128]
        xo_sb = cs_sb[:, 0:W]            # rows {xs, ones}
        onesr_sb = cs_sb[0:1, W:W + P]   # ones row [1, 128]
        oy_sb = cb_sb                    # bf16 interleaved ones/ys

        # DRAM view of t as gather rows [NSL*128*16, 128]
        tg_d = t_d.rearrange("s p (c w) -> (s p c) w", w=CHW)

        # ---- accumulators across slices ----
        # stats2: [:,0]=diag(m2^T m2) sum, [:,1]=diag(m2^T l) sum,
        # [:,2:18] = per-slice per-partition sum(m2)
        stats2 = singles.tile([P, 2 + NSL], F32)
        # per-slice max8/max_index outputs, written directly at stride 8
        pmax8_all = singles.tile([P, 8 * NSL], F32)
        cix_all = singles.tile([P, 8 * NSL], U32)
        wix_all = singles.tile([P, 8 * NSL], U32)
        pxpy_all = singles.tile([2, NSL], F32)
        # whole-run PSUM gram accumulator: [:,0:128]=m2^T m2, [:,128:256]=m2^T l
        both_ps = pboth.tile([P, 2 * P], F32)

        # Per-slice tile handles carried across pipeline stages.
        xs_t, ts_t, es_t, invs_t, cols_t, mls_t, wins_t, mxc_t = (
            {} for _ in range(8))

        def st_load(s):
            x_sb = xp.tile([P, FD], F32, tag="x")
            xs_t[s] = x_sb
            nc.sync.dma_start(out=xs_t[s], in_=x_d[s])
            t_sb = tp.tile([P, FD], F32, tag="t")
            ts_t[s] = t_sb
            nc.sync.dma_start(out=ts_t[s], in_=t_d[s])

        def st_exps(s):
            # exp + S + 1/S broadcast + e col sums. Emitted one slice ahead
            # of st_main so no engine queue waits on same-iteration results.
            e_sb = ep.tile([P, FD], BF16, tag="e")
            es_t[s] = e_sb
            rowe = sm.tile([P, 1], F32, tag="rowe")
            nc.scalar.activation(out=e_sb, in_=xs_t[s], func=ACTF.Exp,
                                 accum_out=rowe)
            sb_ps = psb.tile([P, 1], F32, tag="sb_ps")
            nc.tensor.matmul(sb_ps[:, 0:1], lhsT=ones128_sb,
                             rhs=rowe[:, 0:1], start=True, stop=True)
            invs_sb = sm.tile([P, 1], F32, tag="invs_sb")
            nc.vector.reciprocal(out=invs_sb, in_=sb_ps[:, 0:1])
            invs_t[s] = invs_sb
            cols2 = pcols.tile([2, W], F32, tag="cols")
            cols_t[s] = cols2
            for j in range(NSUB):
                nc.tensor.matmul(
                    cols2[0:2, :], lhsT=oy_sb[:, 2 * j:2 * j + 2],
                    rhs=e_sb[:, j * SUB:(j + 1) * SUB],
                    start=(j == 0), stop=(j == NSUB - 1))

        def st_main(s):
            t_sb = ts_t[s]
            invs_sb = invs_t[s]
            # ---- m2 = e*invS + t (bf16), accum -> sum(m2) ----
            ml_sb = mlp.tile([P, 2 * FD], BF16, tag="ml")
            mls_t[s] = ml_sb
            m2_v = ml_sb[:, 0:FD]
            l_v = ml_sb[:, FD:2 * FD]
            nc.vector.scalar_tensor_tensor(
                out=m2_v, in0=es_t[s], scalar=invs_sb[:, 0:1], in1=t_sb,
                op0=ALU.mult, op1=ALU.add,
                accum_out=stats2[:, 2 + s:3 + s])
            # ---- l = ln(m2) ----
            nc.scalar.activation(out=l_v, in_=m2_v, func=ACTF.Ln)
            # ---- gram-diag: accumulate m2^T[m2|l] blocks into both_ps ----
            ml3 = ml_sb.rearrange("p (c a b) -> p c a b", c=2, b=P)
            for jb in range(NBLK):
                nc.tensor.matmul(
                    both_ps[:, :], lhsT=m2_v[:, jb * P:(jb + 1) * P],
                    rhs=ml3[:, :, jb, :],
                    start=(s == 0 and jb == 0),
                    stop=(s == NSL - 1 and jb == NBLK - 1),
                    skip_group_check=True)

            # ---- chunked argmax of target (exact, first occurrence) ----
            cm = sm.tile([P, NCH], F32, tag="cm")
            nc.vector.tensor_reduce(
                out=cm, in_=t_sb.rearrange("p (a b) -> p a b", b=CHW),
                axis=AX.X, op=ALU.max)
            mx8 = pmax8_all[:, 8 * s:8 * s + 8]
            nc.vector.max(out=mx8, in_=cm)
            cix = cix_all[:, 8 * s:8 * s + 8]
            nc.vector.max_index(out=cix, in_max=mx8, in_values=cm)
            # gather row = cix + s*2048 + p*16 (computed f32, written u32)
            offs_u = sm.tile([P, 1], U32, tag="offs_u")
            nc.vector.scalar_tensor_tensor(
                out=offs_u, in0=cix[:, 0:1], scalar=float(s * P * NCH),
                in1=p16_sb, op0=ALU.add, op1=ALU.add)
            win = winp.tile([P, CHW], F32, tag="win")
            wins_t[s] = win
            nc.gpsimd.indirect_dma_start(
                out=win, out_offset=None, in_=tg_d,
                in_offset=IndirectOffsetOnAxis(ap=offs_u, axis=0))
            # private copy of the max values for st_fix: avoids a
            # cross-iteration WAR on pmax8_all (whole-tile dep tracking)
            mxc = sm.tile([P, 8], F32, tag="mxc")
            nc.vector.tensor_copy(out=mxc, in_=mx8)
            mxc_t[s] = mxc
        def st_px(s):
            # px (row 0) and py (row 1) in one fused [2, 512] dot, with the
            # 1/S scaling folded in (reads PSUM, so DVE)
            pxscr = sm.tile([2, W], F32, tag="pxscr")
            nc.vector.scalar_tensor_tensor(
                out=pxscr, in0=cols_t[s][0:2, :], scalar=invs_t[s][0:2, 0:1],
                in1=xo_sb, op0=ALU.mult, op1=ALU.mult,
                accum_out=pxpy_all[0:2, s:s + 1])

        def st_fix(s):
            # two iterations after the gather was dispatched: no DVE stall
            wix = wix_all[:, 8 * s:8 * s + 8]
            nc.vector.max_index(out=wix, in_max=mxc_t[s],
                                in_values=wins_t[s])

        for k in range(NSL + 6):
            if k < NSL:
                st_load(k)
                st_exps(k)
            if 0 <= k - 1 < NSL:
                st_px(k - 1)
            if 0 <= k - 4 < NSL:
                st_main(k - 4)
            if 0 <= k - 6 < NSL:
                st_fix(k - 6)

        # ================= end-of-loop combine =================
        fin = singles

        # gram diagonals: stats2[:,0] = sum_c m2g[p,p], [:,1] = m2l[p,p]
        dscr = fin.tile([P, 2 * P], F32)
        nc.vector.scalar_tensor_tensor(
            out=dscr[:, 0:P], in0=both_ps[:, 0:P], scalar=1.0, in1=eye_sb,
            op0=ALU.mult, op1=ALU.mult, accum_out=stats2[:, 0:1])
        nc.vector.scalar_tensor_tensor(
            out=dscr[:, P:2 * P], in0=both_ps[:, P:2 * P], scalar=1.0,
            in1=eye_sb, op0=ALU.mult, op1=ALU.mult,
            accum_out=stats2[:, 1:2])

        # cross-partition sums of all per-partition stats in one matmul
        sums_ps = pbig.tile([1, 2 + NSL], F32)
        nc.tensor.matmul(sums_ps[0:1, :], lhsT=ones_sb[:, 0:1],
                         rhs=stats2, start=True, stop=True)

        # per-slice views of the argmax outputs (column 0 of each group of 8)
        pmax_all = pmax8_all.rearrange("p (s k) -> p s k", k=8)[:, :, 0]
        cix_v = cix_all.rearrange("p (s k) -> p s k", k=8)[:, :, 0]
        wix_v = wix_all.rearrange("p (s k) -> p s k", k=8)[:, :, 0]
        # flat = cix*128 + wix + p*2048
        flat_g = fin.tile([P, NSL], F32)
        nc.vector.scalar_tensor_tensor(
            out=flat_g, in0=cix_v, scalar=float(CHW), in1=wix_v,
            op0=ALU.mult, op1=ALU.add)
        flatpb = fin.tile([P, NSL], F32)
        nc.vector.tensor_scalar(out=flatpb, in0=flat_g, scalar1=pb_sb,
                                scalar2=None, op0=ALU.add)

        # py_u lives on partition 1 of pxpy_all; hop it to partition 0
        pyu_row = fin.tile([1, NSL], F32)
        nc.sync.dma_start(out=pyu_row, in_=pxpy_all[1:2, :])

        # per-slice global max via PE transpose
        pmaxT = pcols.tile([NSL, P], F32, tag="cols")
        nc.tensor.transpose(pmaxT[0:NSL, :], pmax_all, eye_sb)
        m_col = fin.tile([NSL, 1], F32)
        nc.vector.reduce_max(out=m_col, in_=pmaxT[0:NSL, :], axis=AX.X)
        m_row = pcols.tile([1, NSL], F32, tag="cols")
        nc.tensor.transpose(m_row[0:1, :], m_col, eye_sb[0:NSL, 0:NSL])
        m_row_sb = fin.tile([1, NSL], F32)
        nc.vector.tensor_copy(out=m_row_sb, in_=m_row[0:1, :])
        m_rep = pcols.tile([P, NSL], F32, tag="cols")
        nc.tensor.matmul(m_rep[:, :], lhsT=onesr_sb[0:1, :],
                         rhs=m_row_sb, start=True, stop=True)
        mk = fin.tile([P, NSL], F32)
        nc.vector.tensor_tensor(out=mk, in0=pmax_all, in1=m_rep[:, :],
                                op=ALU.is_lt)
        # first occurrence = min(flat + 1e9*mk); realized as
        # max(-(flat + 1e9*mk)) = max((mk * -1e9) - flat)
        fneg = fin.tile([P, NSL], F32)
        nc.vector.scalar_tensor_tensor(
            out=fneg, in0=mk, scalar=-1.0e9, in1=flatpb,
            op0=ALU.mult, op1=ALU.subtract)
        fnegT = pcols.tile([NSL, P], F32, tag="cols")
        nc.tensor.transpose(fnegT[0:NSL, :], fneg, eye_sb)
        fmax_col = fin.tile([NSL, 1], F32)
        nc.vector.reduce_max(out=fmax_col, in_=fnegT[0:NSL, :], axis=AX.X)
        fmin_col = fin.tile([NSL, 1], F32)
        nc.vector.tensor_scalar(out=fmin_col, in0=fmax_col, scalar1=-1.0,
                                scalar2=None, op0=ALU.mult)
        f_row = pcols.tile([1, NSL], F32, tag="cols")
        nc.tensor.transpose(f_row[0:1, :], fmin_col, eye_sb[0:NSL, 0:NSL])
        F_sb = fin.tile([1, NSL], F32)
        nc.vector.tensor_copy(out=F_sb, in_=f_row[0:1, :])

        # decompose flat -> (h, w); tx = (w+1)/W, ty = (h+1)/H
        Fi = fin.tile([1, NSL], I32)
        nc.vector.tensor_copy(out=Fi, in_=F_sb)
        wi = fin.tile([1, NSL], I32)
        nc.vector.tensor_scalar(out=wi, in0=Fi, scalar1=W - 1,
                                scalar2=None, op0=ALU.bitwise_and)
        hi = fin.tile([1, NSL], I32)
        nc.vector.tensor_scalar(out=hi, in0=Fi, scalar1=9,
                                scalar2=None, op0=ALU.arith_shift_right)
        wf = fin.tile([1, NSL], F32)
        nc.vector.tensor_copy(out=wf, in_=wi)
        hf = fin.tile([1, NSL], F32)
        nc.vector.tensor_copy(out=hf, in_=hi)
        tx = fin.tile([1, NSL], F32)
        nc.vector.tensor_scalar(out=tx, in0=wf, scalar1=1.0,
                                scalar2=1.0 / W, op0=ALU.add, op1=ALU.mult)
        ty = fin.tile([1, NSL], F32)
        nc.vector.tensor_scalar(out=ty, in0=hf, scalar1=1.0,
                                scalar2=1.0 / H, op0=ALU.add, op1=ALU.mult)

        # px, py (the 1/S scaling was folded into the pxscr pass)
        px = pxpy_all[0:1, :]
        py = pyu_row

        # ed = sqrt((tx-px)^2 + (ty-py)^2), summed
        dx = fin.tile([1, NSL], F32)
        nc.vector.tensor_tensor(out=dx, in0=tx, in1=px, op=ALU.subtract)
        dy = fin.tile([1, NSL], F32)
        nc.vector.tensor_tensor(out=dy, in0=ty, in1=py, op=ALU.subtract)
        d2 = fin.tile([1, NSL], F32)
        nc.vector.tensor_tensor(out=d2, in0=dx, in1=dx, op=ALU.mult)
        d2b = fin.tile([1, NSL], F32)
        nc.vector.tensor_tensor(out=d2b, in0=dy, in1=dy, op=ALU.mult)
        ed2 = fin.tile([1, NSL], F32)
        nc.vector.tensor_tensor(out=ed2, in0=d2, in1=d2b, op=ALU.add)

        # pair (c=0 vs c=1) distances, pred and true
        NP2 = NSL // 2
        def pairs(v):
            r = v[0:1, :].rearrange("p (b c) -> p b c", c=2)
            return r[:, :, 0:1], r[:, :, 1:2]

        px0, px1 = pairs(px)
        py0, py1 = pairs(py)
        tx0, tx1 = pairs(tx)
        ty0, ty1 = pairs(ty)
        dpx = fin.tile([1, NP2, 1], F32)
        nc.vector.tensor_tensor(out=dpx, in0=px0, in1=px1, op=ALU.subtract)
        dpy = fin.tile([1, NP2, 1], F32)
        nc.vector.tensor_tensor(out=dpy, in0=py0, in1=py1, op=ALU.subtract)
        dtx = fin.tile([1, NP2, 1], F32)
        nc.vector.tensor_tensor(out=dtx, in0=tx0, in1=tx1, op=ALU.subtract)
        dty = fin.tile([1, NP2, 1], F32)
        nc.vector.tensor_tensor(out=dty, in0=ty0, in1=ty1, op=ALU.subtract)
        pd2 = fin.tile([1, NP2, 1], F32)
        nc.vector.tensor_tensor(out=pd2, in0=dpx, in1=dpx, op=ALU.mult)
        pd2b = fin.tile([1, NP2, 1], F32)
        nc.vector.tensor_tensor(out=pd2b, in0=dpy, in1=dpy, op=ALU.mult)
        nc.vector.tensor_tensor(out=pd2, in0=pd2, in1=pd2b, op=ALU.add)
        td2 = fin.tile([1, NP2, 1], F32)
        nc.vector.tensor_tensor(out=td2, in0=dtx, in1=dtx, op=ALU.mult)
        td2b = fin.tile([1, NP2, 1], F32)
        nc.vector.tensor_tensor(out=td2b, in0=dty, in1=dty, op=ALU.mult)
        nc.vector.tensor_tensor(out=td2, in0=td2, in1=td2b, op=ALU.add)

        # sqrts grouped (single act-table switch)
        ed = fin.tile([1, NSL], F32)
        nc.scalar.activation(out=ed, in_=ed2, func=ACTF.Sqrt)
        pd = fin.tile([1, NP2, 1], F32)
        nc.scalar.activation(out=pd, in_=pd2, func=ACTF.Sqrt)
        td = fin.tile([1, NP2, 1], F32)
        nc.scalar.activation(out=td, in_=td2, func=ACTF.Sqrt)

        eds = fin.tile([1, 1], F32)
        nc.vector.reduce_sum(out=eds, in_=ed, axis=AX.X)
        dd = fin.tile([1, NP2, 1], F32)
        nc.vector.tensor_tensor(out=dd, in0=pd, in1=td, op=ALU.subtract)
        dsum = fin.tile([1, 1], F32)
        nc.vector.tensor_reduce(out=dsum, in_=dd, axis=AX.XY, op=ALU.add,
                                apply_absolute_value=True)

        # jsd total = [0.5*W1 - 0.5*ln2*W0 - 0.25*W2] / (H*W)
        # sums_ps row: [0]=W2, [1]=W1, [2:18]=per-slice sum(m2)
        sums_sb = fin.tile([1, 2 + NSL], F32)
        nc.vector.tensor_copy(out=sums_sb, in_=sums_ps[0:1, :])
        w0 = fin.tile([1, 1], F32)
        nc.vector.reduce_sum(out=w0, in_=sums_sb[0:1, 2:2 + NSL], axis=AX.X)
        j1 = fin.tile([1, 1], F32)
        nc.vector.scalar_tensor_tensor(
            out=j1, in0=w0, scalar=-math.log(2.0), in1=sums_sb[0:1, 1:2],
            op0=ALU.mult, op1=ALU.add)
        j2 = fin.tile([1, 1], F32)
        nc.vector.scalar_tensor_tensor(
            out=j2, in0=sums_sb[0:1, 0:1], scalar=-0.5, in1=j1,
            op0=ALU.mult, op1=ALU.add)
        stot = fin.tile([1, 1], F32)
        nc.vector.scalar_tensor_tensor(
            out=stot, in0=j2, scalar=0.5 / float(H * W), in1=eds,
            op0=ALU.mult, op1=ALU.add)
        nc.vector.tensor_tensor(out=stot, in0=stot, in1=dsum, op=ALU.add)

        nc.sync.dma_start(out=out_d[0:1, 0:1], in_=stot)


def make_in_maps(input, target):
    consts = _constants()
    in_maps = []
    for i in range(N_CORES):
        xs = np.ascontiguousarray(
            input[i * B_SH:(i + 1) * B_SH].reshape(NSL, P, FD))
        ts = np.ascontiguousarray(
            target[i * B_SH:(i + 1) * B_SH].reshape(NSL, P, FD))
        m = {"x": xs, "t": ts}
        m.update(consts)
        in_maps.append(m)
    return in_maps


def kernel(input, target):
    global LAST_RESULTS
    input = np.asarray(input, dtype=np.float32)
    target = np.asarray(target, dtype=np.float32)
    nc = build_program()
    in_maps = make_in_maps(input, target)
    res = run_bass_kernel_spmd(nc, in_maps, list(range(N_CORES)))
    LAST_RESULTS = res
    s = 0.0
    for i in range(N_CORES):
        s += float(res.results[i]["out"][0, 0])
    return np.array([s / B], dtype=np.float32)


# revision 27
# speedup vs baseline: 1.0878x; 1.0878x over previous
"""DSNT + JSD + distance double loss on 8 TRN2 NeuronCores.

Data-parallel: batch 64 is split into 8 shards of 8 batches; each core
computes its partial sum s_i over its 16 (b,c) heatmap slices; the host
sums the 8 partials and divides by B.

Per (b,c) slice (512x512 -> SBUF [128, 2048], partition p holds rows
h in {4p..4p+3}):
  ACT:  e = exp(x) (bf16), accum -> per-partition rowsums -> S, 1/S
  PE:   cols2 = [ones; ys]^T @ e  (bf16)  -> px_u, py_u via Pool stt
  DVE/Pool: m2 = (e*invS) + t (bf16), accum -> sum(m2)
  ACT:  l = ln(m2) (bf16), written into the same tile as m2
  PE:   gram-diag trick: for each 128-col block, matmul with
        lhsT=m2_block, rhs=[m2_block | l_block] accumulated into one
        whole-run PSUM [128,256]; its diagonals give sum(m2^2), sum(m2*l)
  argmax(t) chunked: DVE chunk-max [P,16,128]->cm, max8+max_index on cm
        -> first chunk; indirect DMA gathers that 128-wide window from
        DRAM; max_index on the window -> exact first-occurrence index.
jsd total = [0.5*W1 - 0.5*ln2*W0 - 0.25*W2] / (H*W)
"""

import math
import os

import numpy as np

import concourse.bacc as bacc
import concourse.bass as bass
import concourse.mybir as mybir
import concourse.tile as tile
from concourse.bass import IndirectOffsetOnAxis
from concourse.bass_utils import run_bass_kernel_spmd

F32 = mybir.dt.float32
BF16 = mybir.dt.bfloat16
U32 = mybir.dt.uint32
I32 = mybir.dt.int32
ALU = mybir.AluOpType
ACTF = mybir.ActivationFunctionType
AX = mybir.AxisListType

B, C, H, W = 64, 2, 512, 512
N_CORES = 8
B_SH = B // N_CORES          # 8 batches per core
NSL = B_SH * C               # 16 slices per core
P = 128                      # SBUF partitions
FD = (H * W) // P            # 2048 free elements per partition
SUB = W                      # 512-wide sub-columns (4 per row)
NSUB = FD // SUB             # 4
NCH = 16                     # argmax chunks per partition
CHW = FD // NCH              # 128 elements per chunk
NBLK = FD // P               # 16 gram blocks of 128 columns

# Input DMA buffer depth.
IN_BUFS = int(os.environ.get("K_IN_BUFS", "5"))

_CACHE = {}
LAST_RESULTS = None


def _constants():
    # cf32 [128, 260]: col0 = p*2048 (flat partition base), col1 = ones,
    # cols 2:130 = identity, col130 = p*16 (gather row base),
    # cols 132:260 = all-ones [128,128] (S broadcast matmul lhsT)
    cf32 = np.zeros((P, 260), dtype=np.float32)
    cf32[:, 0] = np.arange(P, dtype=np.float32) * FD
    cf32[:, 1] = 1.0
    cf32[:, 2:130] = np.eye(P, dtype=np.float32)
    cf32[:, 130] = np.arange(P, dtype=np.float32) * NCH
    cf32[:, 132:260] = 1.0
    # csml [2, 640]: [:, 0:512] = {xs row, ones row}; [0, 512:640] = onesr
    xs = (np.arange(W, dtype=np.float32) + 1.0) / W
    csml = np.zeros((2, 640), dtype=np.float32)
    csml[0, 0:W] = xs
    csml[1, 0:W] = 1.0
    csml[0, W:W + P] = 1.0
    # cbf [128, 8] bf16: col 2j = ones, col 2j+1 = ys_j ((4p+j+1)/512)
    import ml_dtypes
    hidx = (np.arange(P, dtype=np.float32)[:, None] * NSUB
            + np.arange(NSUB, dtype=np.float32)[None, :])
    ys = (hidx + 1.0) / H                                   # [128, 4]
    cbf = np.zeros((P, 8), dtype=np.float32)
    for j in range(NSUB):
        cbf[:, 2 * j] = 1.0
        cbf[:, 2 * j + 1] = ys[:, j]
    return {"cf32": cf32, "csml": csml,
            "cbf": cbf.astype(ml_dtypes.bfloat16)}


def _patch_act_tables():
    """Steer the act-table chooser so Exp/Ln/Copy/Identity all live in the
    single `natural_log_exp_and_others` set — otherwise the per-slice
    Exp->Ln alternation reloads tables (~1.3us each, 32x per core)."""
    if _CACHE.get("act_patched"):
        return
    import concourse.hw_specs as hw_specs

    orig = hw_specs.get_activation_tables
    hot = {ACTF.Exp, ACTF.Ln, ACTF.Square, ACTF.Copy, ACTF.Identity}

    def patched(module_arch):
        tabs = orig(module_arch)
        out = {}
        for name, funcs in tabs.items():
            if name == "natural_log_exp_and_others":
                out[name] = set(funcs)
            else:
                out[name] = set(funcs) - hot
        return out

    hw_specs.get_activation_tables = patched
    bacc.get_activation_tables = patched
    _CACHE["act_patched"] = True


def build_program():
    """Build (once) the single-core Bass/Tile program run SPMD on 8 cores."""
    if "nc" in _CACHE:
        return _CACHE["nc"]

    _patch_act_tables()
    nc = bacc.Bacc("TRN2", target_bir_lowering=False, debug=False,
                   num_devices=N_CORES)

    x_t = nc.dram_tensor("x", [NSL, P, FD], F32, kind="ExternalInput")
    t_t = nc.dram_tensor("t", [NSL, P, FD], F32, kind="ExternalInput")
    cf_d = nc.dram_tensor("cf32", [P, 260], F32, kind="ExternalInput").ap()
    cs_d = nc.dram_tensor("csml", [2, 640], F32, kind="ExternalInput").ap()
    cb_d = nc.dram_tensor("cbf", [P, 8], BF16, kind="ExternalInput").ap()
    out_d = nc.dram_tensor("out", [1, 1], F32, kind="ExternalOutput").ap()

    with tile.TileContext(nc) as tc:
        _emit(nc, tc, x_t.ap(), t_t.ap(), cf_d, cs_d, cb_d, out_d)

    nc.compile()
    _CACHE["nc"] = nc
    return nc


def _emit(nc, tc, x_d, t_d, cf_d, cs_d, cb_d, out_d):
    from contextlib import ExitStack
    ctx = ExitStack()
    with ctx:
        singles = ctx.enter_context(tc.tile_pool(name="singles", bufs=1))
        xp = ctx.enter_context(tc.tile_pool(name="xp", bufs=IN_BUFS + 1))
        tp = ctx.enter_context(tc.tile_pool(name="tp", bufs=IN_BUFS + 2))
        ep = ctx.enter_context(tc.tile_pool(name="ep", bufs=6))
        mlp = ctx.enter_context(tc.tile_pool(name="mlp", bufs=4))
        winp = ctx.enter_context(tc.tile_pool(name="winp", bufs=NSL))
        sm = ctx.enter_context(tc.tile_pool(name="sm", bufs=8))
        pcols = ctx.enter_context(
            tc.tile_pool(name="pcols", bufs=3, space="PSUM"))
        pboth = ctx.enter_context(
            tc.tile_pool(name="pboth", bufs=1, space="PSUM"))
        psb = ctx.enter_context(
            tc.tile_pool(name="psb", bufs=2, space="PSUM"))
        pbig = ctx.enter_context(
            tc.tile_pool(name="pbig", bufs=1, space="PSUM"))

        # ---- constants: 3 packed DMAs on the ACT queue ----
        cf_sb = singles.tile([P, 260], F32)
        nc.scalar.dma_start(out=cf_sb, in_=cf_d)
        cs_sb = singles.tile([2, 640], F32)
        nc.scalar.dma_start(out=cs_sb, in_=cs_d)
        cb_sb = singles.tile([P, 8], BF16)
        nc.scalar.dma_start(out=cb_sb, in_=cb_d)
        pb_sb = cf_sb[:, 0:1]            # p*2048
        ones_sb = cf_sb[:, 1:2]          # 1.0 column (f32)
        eye_sb = cf_sb[:, 2:2 + P]       # identity
        p16_sb = cf_sb[:, 130:131]       # p*16
        ones128_sb = cf_sb[:, 132:260]   # all-ones [128,128]
        xo_sb = cs_sb[:, 0:W]            # rows {xs, ones}
        onesr_sb = cs_sb[0:1, W:W + P]   # ones row [1, 128]
        oy_sb = cb_sb                    # bf16 interleaved ones/ys

        # DRAM view of t as gather rows [NSL*128*16, 128]
        tg_d = t_d.rearrange("s p (c w) -> (s p c) w", w=CHW)

        # ---- accumulators across slices ----
        # stats2: [:,0]=diag(m2^T m2) sum, [:,1]=diag(m2^T l) sum,
        # [:,2:18] = per-slice per-partition sum(m2)
        stats2 = singles.tile([P, 2 + NSL], F32)
        # per-slice max8/max_index outputs, written directly at stride 8
        pmax8_all = singles.tile([P, 8 * NSL], F32)
        cix_all = singles.tile([P, 8 * NSL], U32)
        wix_all = singles.tile([P, 8 * NSL], U32)
        pxpy_all = singles.tile([2, NSL], F32)
        # whole-run PSUM gram accumulator: [:,0:128]=m2^T m2, [:,128:256]=m2^T l
        both_ps = pboth.tile([P, 2 * P], F32)

        # Per-slice tile handles carried across pipeline stages.
        xs_t, ts_t, es_t, invs_t, cols_t, mls_t, wins_t = (
            {} for _ in range(7))

        def st_load(s):
            x_sb = xp.tile([P, FD], F32, tag="x")
            xs_t[s] = x_sb
            nc.sync.dma_start(out=xs_t[s], in_=x_d[s])
            t_sb = tp.tile([P, FD], F32, tag="t")
            ts_t[s] = t_sb
            nc.sync.dma_start(out=ts_t[s], in_=t_d[s])

        def st_exps(s):
            # exp + S + 1/S broadcast + e col sums. Emitted one slice ahead
            # of st_main so no engine queue waits on same-iteration results.
            e_sb = ep.tile([P, FD], BF16, tag="e")
            es_t[s] = e_sb
            rowe = sm.tile([P, 1], F32, tag="rowe")
            nc.scalar.activation(out=e_sb, in_=xs_t[s], func=ACTF.Exp,
                                 accum_out=rowe)
            sb_ps = psb.tile([P, 1], F32, tag="sb_ps")
            nc.tensor.matmul(sb_ps[:, 0:1], lhsT=ones128_sb,
                             rhs=rowe[:, 0:1], start=True, stop=True)
            invs_sb = sm.tile([P, 1], F32, tag="invs_sb")
            nc.vector.reciprocal(out=invs_sb, in_=sb_ps[:, 0:1])
            invs_t[s] = invs_sb
            cols2 = pcols.tile([2, W], F32, tag="cols")
            cols_t[s] = cols2
            for j in range(NSUB):
                nc.tensor.matmul(
                    cols2[0:2, :], lhsT=oy_sb[:, 2 * j:2 * j + 2],
                    rhs=e_sb[:, j * SUB:(j + 1) * SUB],
                    start=(j == 0), stop=(j == NSUB - 1))

        def st_main(s):
            t_sb = ts_t[s]
            invs_sb = invs_t[s]
            # ---- m2 = e*invS + t (bf16), accum -> sum(m2) ----
            ml_sb = mlp.tile([P, 2 * FD], BF16, tag="ml")
            mls_t[s] = ml_sb
            m2_v = ml_sb[:, 0:FD]
            l_v = ml_sb[:, FD:2 * FD]
            nc.vector.scalar_tensor_tensor(
                out=m2_v, in0=es_t[s], scalar=invs_sb[:, 0:1], in1=t_sb,
                op0=ALU.mult, op1=ALU.add,
                accum_out=stats2[:, 2 + s:3 + s])

            # ---- chunked argmax of target (exact, first occurrence) ----
            cm = sm.tile([P, NCH], F32, tag="cm")
            nc.vector.tensor_reduce(
                out=cm, in_=t_sb.rearrange("p (a b) -> p a b", b=CHW),
                axis=AX.X, op=ALU.max)
            mx8 = pmax8_all[:, 8 * s:8 * s + 8]
            nc.vector.max(out=mx8, in_=cm)
            cix = cix_all[:, 8 * s:8 * s + 8]
            nc.vector.max_index(out=cix, in_max=mx8, in_values=cm)
            # gather row = cix + s*2048 + p*16 (computed f32, written u32)
            offs_u = sm.tile([P, 1], U32, tag="offs_u")
            nc.vector.scalar_tensor_tensor(
                out=offs_u, in0=cix[:, 0:1], scalar=float(s * P * NCH),
                in1=p16_sb, op0=ALU.add, op1=ALU.add)
            win = winp.tile([P, CHW], F32, tag="win")
            wins_t[s] = win
            nc.gpsimd.indirect_dma_start(
                out=win, out_offset=None, in_=tg_d,
                in_offset=IndirectOffsetOnAxis(ap=offs_u, axis=0))

        def st_ln(s):
            # one iteration after m2 so the ACT queue never waits on the DVE
            ml_sb = mls_t[s]
            m2_v = ml_sb[:, 0:FD]
            l_v = ml_sb[:, FD:2 * FD]
            nc.scalar.activation(out=l_v, in_=m2_v, func=ACTF.Ln)
            # ---- gram-diag: accumulate m2^T[m2|l] blocks into both_ps ----
            ml3 = ml_sb.rearrange("p (c a b) -> p c a b", c=2, b=P)
            for jb in range(NBLK):
                nc.tensor.matmul(
                    both_ps[:, :], lhsT=m2_v[:, jb * P:(jb + 1) * P],
                    rhs=ml3[:, :, jb, :],
                    start=(s == 0 and jb == 0),
                    stop=(s == NSL - 1 and jb == NBLK - 1),
                    skip_group_check=True)

        def st_px(s):
            # px (row 0) and py (row 1) in one fused [2, 512] dot, with the
            # 1/S scaling folded in (reads PSUM, so DVE)
            pxscr = sm.tile([2, W], F32, tag="pxscr")
            nc.vector.scalar_tensor_tensor(
                out=pxscr, in0=cols_t[s][0:2, :], scalar=invs_t[s][0:2, 0:1],
                in1=xo_sb, op0=ALU.mult, op1=ALU.mult,
                accum_out=pxpy_all[0:2, s:s + 1])

        for k in range(NSL + 5):
            if k < NSL:
                st_load(k)
                st_exps(k)
            if 0 <= k - 1 < NSL:
                st_px(k - 1)
            if 0 <= k - 4 < NSL:
                st_main(k - 4)
            if 0 <= k - 5 < NSL:
                st_ln(k - 5)

        # all gathers are long done: batch the window max_index lookups
        for s in range(NSL):
            wixv = wix_all[:, 8 * s:8 * s + 8]
            nc.vector.max_index(out=wixv,
                                in_max=pmax8_all[:, 8 * s:8 * s + 8],
                                in_values=wins_t[s])

        # ================= end-of-loop combine =================
        fin = singles

        # gram diagonals: stats2[:,0] = sum_c m2g[p,p], [:,1] = m2l[p,p]
        dscr = fin.tile([P, 2 * P], F32)
        nc.vector.scalar_tensor_tensor(
            out=dscr[:, 0:P], in0=both_ps[:, 0:P], scalar=1.0, in1=eye_sb,
            op0=ALU.mult, op1=ALU.mult, accum_out=stats2[:, 0:1])
        nc.vector.scalar_tensor_tensor(
            out=dscr[:, P:2 * P], in0=both_ps[:, P:2 * P], scalar=1.0,
            in1=eye_sb, op0=ALU.mult, op1=ALU.mult,
            accum_out=stats2[:, 1:2])

        # cross-partition sums of all per-partition stats in one matmul
        sums_ps = pbig.tile([1, 2 + NSL], F32)
        nc.tensor.matmul(sums_ps[0:1, :], lhsT=ones_sb[:, 0:1],
                         rhs=stats2, start=True, stop=True)

        # per-slice views of the argmax outputs (column 0 of each group of 8)
        pmax_all = pmax8_all.rearrange("p (s k) -> p s k", k=8)[:, :, 0]
        cix_v = cix_all.rearrange("p (s k) -> p s k", k=8)[:, :, 0]
        wix_v = wix_all.rearrange("p (s k) -> p s k", k=8)[:, :, 0]
        # flat = cix*128 + wix + p*2048
        flat_g = fin.tile([P, NSL], F32)
        nc.vector.scalar_tensor_tensor(
            out=flat_g, in0=cix_v, scalar=float(CHW), in1=wix_v,
            op0=ALU.mult, op1=ALU.add)
        flatpb = fin.tile([P, NSL], F32)
        nc.vector.tensor_scalar(out=flatpb, in0=flat_g, scalar1=pb_sb,
                                scalar2=None, op0=ALU.add)

        # py_u lives on partition 1 of pxpy_all; hop it to partition 0
        pyu_row = fin.tile([1, NSL], F32)
        nc.sync.dma_start(out=pyu_row, in_=pxpy_all[1:2, :])

        # per-slice global max via PE transpose
        pmaxT = pcols.tile([NSL, P], F32, tag="cols")
        nc.tensor.transpose(pmaxT[0:NSL, :], pmax_all, eye_sb)
        m_col = fin.tile([NSL, 1], F32)
        nc.vector.reduce_max(out=m_col, in_=pmaxT[0:NSL, :], axis=AX.X)
        m_row = pcols.tile([1, NSL], F32, tag="cols")
        nc.tensor.transpose(m_row[0:1, :], m_col, eye_sb[0:NSL, 0:NSL])
        m_row_sb = fin.tile([1, NSL], F32)
        nc.vector.tensor_copy(out=m_row_sb, in_=m_row[0:1, :])
        m_rep = pcols.tile([P, NSL], F32, tag="cols")
        nc.tensor.matmul(m_rep[:, :], lhsT=onesr_sb[0:1, :],
                         rhs=m_row_sb, start=True, stop=True)
        mk = fin.tile([P, NSL], F32)
        nc.vector.tensor_tensor(out=mk, in0=pmax_all, in1=m_rep[:, :],
                                op=ALU.is_lt)
        # first occurrence = min(flat + 1e9*mk); realized as
        # max(-(flat + 1e9*mk)) = max((mk * -1e9) - flat)
        fneg = fin.tile([P, NSL], F32)
        nc.vector.scalar_tensor_tensor(
            out=fneg, in0=mk, scalar=-1.0e9, in1=flatpb,
            op0=ALU.mult, op1=ALU.subtract)
        fnegT = pcols.tile([NSL, P], F32, tag="cols")
        nc.tensor.transpose(fnegT[0:NSL, :], fneg, eye_sb)
        fmax_col = fin.tile([NSL, 1], F32)
        nc.vector.reduce_max(out=fmax_col, in_=fnegT[0:NSL, :], axis=AX.X)
        fmin_col = fin.tile([NSL, 1], F32)
        nc.vector.tensor_scalar(out=fmin_col, in0=fmax_col, scalar1=-1.0,
                                scalar2=None, op0=ALU.mult)
        f_row = pcols.tile([1, NSL], F32, tag="cols")
        nc.tensor.transpose(f_row[0:1, :], fmin_col, eye_sb[0:NSL, 0:NSL])
        F_sb = fin.tile([1, NSL], F32)
        nc.vector.tensor_copy(out=F_sb, in_=f_row[0:1, :])

        # decompose flat -> (h, w); tx = (w+1)/W, ty = (h+1)/H
        Fi = fin.tile([1, NSL], I32)
        nc.vector.tensor_copy(out=Fi, in_=F_sb)
        wi = fin.tile([1, NSL], I32)
        nc.vector.tensor_scalar(out=wi, in0=Fi, scalar1=W - 1,
                                scalar2=None, op0=ALU.bitwise_and)
        hi = fin.tile([1, NSL], I32)
        nc.vector.tensor_scalar(out=hi, in0=Fi, scalar1=9,
                                scalar2=None, op0=ALU.arith_shift_right)
        wf = fin.tile([1, NSL], F32)
        nc.vector.tensor_copy(out=wf, in_=wi)
        hf = fin.tile([1, NSL], F32)
        nc.vector.tensor_copy(out=hf, in_=hi)
        tx = fin.tile([1, NSL], F32)
        nc.vector.tensor_scalar(out=tx, in0=wf, scalar1=1.0,
                                scalar2=1.0 / W, op0=ALU.add, op1=ALU.mult)
        ty = fin.tile([1, NSL], F32)
        nc.vector.tensor_scalar(out=ty, in0=hf, scalar1=1.0,
                                scalar2=1.0 / H, op0=ALU.add, op1=ALU.mult)

        # px, py (the 1/S scaling was folded into the pxscr pass)
        px = pxpy_all[0:1, :]
        py = pyu_row

        # ed = sqrt((tx-px)^2 + (ty-py)^2), summed
        dx = fin.tile([1, NSL], F32)
        nc.vector.tensor_tensor(out=dx, in0=tx, in1=px, op=ALU.subtract)
        dy = fin.tile([1, NSL], F32)
        nc.vector.tensor_tensor(out=dy, in0=ty, in1=py, op=ALU.subtract)
        d2 = fin.tile([1, NSL], F32)
        nc.vector.tensor_tensor(out=d2, in0=dx, in1=dx, op=ALU.mult)
        d2b = fin.tile([1, NSL], F32)
        nc.vector.tensor_tensor(out=d2b, in0=dy, in1=dy, op=ALU.mult)
        ed2 = fin.tile([1, NSL], F32)
        nc.vector.tensor_tensor(out=ed2, in0=d2, in1=d2b, op=ALU.add)

        # pair (c=0 vs c=1) distances, pred and true
        NP2 = NSL // 2
        def pairs(v):
            r = v[0:1, :].rearrange("p (b c) -> p b c", c=2)
            return r[:, :, 0:1], r[:, :, 1:2]

        px0, px1 = pairs(px)
        py0, py1 = pairs(py)
        tx0, tx1 = pairs(tx)
        ty0, ty1 = pairs(ty)
        dpx = fin.tile([1, NP2, 1], F32)
        nc.vector.tensor_tensor(out=dpx, in0=px0, in1=px1, op=ALU.subtract)
        dpy = fin.tile([1, NP2, 1], F32)
        nc.vector.tensor_tensor(out=dpy, in0=py0, in1=py1, op=ALU.subtract)
        dtx = fin.tile([1, NP2, 1], F32)
        nc.vector.tensor_tensor(out=dtx, in0=tx0, in1=tx1, op=ALU.subtract)
        dty = fin.tile([1, NP2, 1], F32)
        nc.vector.tensor_tensor(out=dty, in0=ty0, in1=ty1, op=ALU.subtract)
        pd2 = fin.tile([1, NP2, 1], F32)
        nc.vector.tensor_tensor(out=pd2, in0=dpx, in1=dpx, op=ALU.mult)
        pd2b = fin.tile([1, NP2, 1], F32)
        nc.vector.tensor_tensor(out=pd2b, in0=dpy, in1=dpy, op=ALU.mult)
        nc.vector.tensor_tensor(out=pd2, in0=pd2, in1=pd2b, op=ALU.add)
        td2 = fin.tile([1, NP2, 1], F32)
        nc.vector.tensor_tensor(out=td2, in0=dtx, in1=dtx, op=ALU.mult)
        td2b = fin.tile([1, NP2, 1], F32)
        nc.vector.tensor_tensor(out=td2b, in0=dty, in1=dty, op=ALU.mult)
        nc.vector.tensor_tensor(out=td2, in0=td2, in1=td2b, op=ALU.add)

        # sqrts grouped (single act-table switch)
        ed = fin.tile([1, NSL], F32)
        nc.scalar.activation(out=ed, in_=ed2, func=ACTF.Sqrt)
        pd = fin.tile([1, NP2, 1], F32)
        nc.scalar.activation(out=pd, in_=pd2, func=ACTF.Sqrt)
        td = fin.tile([1, NP2, 1], F32)
        nc.scalar.activation(out=td, in_=td2, func=ACTF.Sqrt)

        eds = fin.tile([1, 1], F32)
        nc.vector.reduce_sum(out=eds, in_=ed, axis=AX.X)
        dd = fin.tile([1, NP2, 1], F32)
        nc.vector.tensor_tensor(out=dd, in0=pd, in1=td, op=ALU.subtract)
        dsum = fin.tile([1, 1], F32)
        nc.vector.tensor_reduce(out=dsum, in_=dd, axis=AX.XY, op=ALU.add,
                                apply_absolute_value=True)

        # jsd total = [0.5*W1 - 0.5*ln2*W0 - 0.25*W2] / (H*W)
        # sums_ps row: [0]=W2, [1]=W1, [2:18]=per-slice sum(m2)
        sums_sb = fin.tile([1, 2 + NSL], F32)
        nc.vector.tensor_copy(out=sums_sb, in_=sums_ps[0:1, :])
        w0 = fin.tile([1, 1], F32)
        nc.vector.reduce_sum(out=w0, in_=sums_sb[0:1, 2:2 + NSL], axis=AX.X)
        j1 = fin.tile([1, 1], F32)
        nc.vector.scalar_tensor_tensor(
            out=j1, in0=w0, scalar=-math.log(2.0), in1=sums_sb[0:1, 1:2],
            op0=ALU.mult, op1=ALU.add)
        j2 = fin.tile([1, 1], F32)
        nc.vector.scalar_tensor_tensor(
            out=j2, in0=sums_sb[0:1, 0:1], scalar=-0.5, in1=j1,
            op0=ALU.mult, op1=ALU.add)
        stot = fin.tile([1, 1], F32)
        nc.vector.scalar_tensor_tensor(
            out=stot, in0=j2, scalar=0.5 / float(H * W), in1=eds,
            op0=ALU.mult, op1=ALU.add)
        nc.vector.tensor_tensor(out=stot, in0=stot, in1=dsum, op=ALU.add)

        nc.sync.dma_start(out=out_d[0:1, 0:1], in_=stot)


def make_in_maps(input, target):
    consts = _constants()
    in_maps = []
    for i in range(N_CORES):
        xs = np.ascontiguousarray(
            input[i * B_SH:(i + 1) * B_SH].reshape(NSL, P, FD))
        ts = np.ascontiguousarray(
            target[i * B_SH:(i + 1) * B_SH].reshape(NSL, P, FD))
        m = {"x": xs, "t": ts}
        m.update(consts)
        in_maps.append(m)
    return in_maps


def kernel(input, target):
    global LAST_RESULTS
    input = np.asarray(input, dtype=np.float32)
    target = np.asarray(target, dtype=np.float32)
    nc = build_program()
    in_maps = make_in_maps(input, target)
    res = run_bass_kernel_spmd(nc, in_maps, list(range(N_CORES)))
    LAST_RESULTS = res
    s = 0.0
    for i in range(N_CORES):
        s += float(res.results[i]["out"][0, 0])
    return np.array([s / B], dtype=np.float32)
